# revision 18
# baseline (speedup 1.0000x reference)
"""Trainium2 Bass kernel for a 6-layer pre-LN transformer encoder (nn_Encoder).

Distribution: tokens sharded 8 ways (core c -> batch c//4, seq chunk c%4 of 512
tokens).  Per layer, each core computes K^T/V projections for its own tokens,
AllGathers them (bf16) within its batch group of 4 cores, and computes
attention for its 512 query tokens over the full 2048-key sequence.
Projections / FFN / LayerNorms are purely local to the token shard.

The residual stream is kept transposed (x.T: [D, tok]) so every matmul
consumes operands in natural layout with no per-layer transposes:
  q.T/k.T = W.T @ h.T    (lhsT = W chunk,   rhs = h.T chunk)
  v       = h @ W        (lhsT = h.T chunk, rhs = W chunk)
  scores.T[k,q]          (lhsT = K.T chunk, rhs = q.T head)  [2 heads row-packed]
  attn.T  = V.T @ P.T    (lhsT = V chunk,   rhs = expP)      [2 heads col-packed]
  Z       = ones.T @ P.T                                     [2 heads col-packed]
  out.T   = WO.T @ attn.T; FFN analogous.
LayerNorm reductions over D (partition axis) use (1/D)-valued ones-matmuls;
per-token scalars are broadcast across partitions with K=1 ones matmuls.
rsqrt(var+eps) = Exp(-0.5 * Ln(var+eps)) keeps ACT on one table set.
The key-padding mask folds into the exp() activation bias (per-partition).

v2 pipeline: everything after attention (WO/LN2/FFN/LN1'/KV'/Q') is processed
in two token halves; the K/V AllGather for layer l+1 is split by token half
and launched at the end of each half's tail work, so the collective overlaps
the other half's FFN + the next attention's first chunks.  Softmax 1/Z uses a
single custom-DVE reciprocal_approx_fast over both packed head rows.
"""

import math

import numpy as np
import ml_dtypes

import concourse.bass as bass
import concourse.mybir as mybir
import concourse.tile as tile
from concourse import bacc
from concourse.bass_utils import run_bass_kernel_spmd

F32 = mybir.dt.float32
BF16 = mybir.dt.bfloat16
I32 = mybir.dt.int32
AF = mybir.ActivationFunctionType
OP = mybir.AluOpType

VOCAB, D, H, L, DFF, PAD = 32000, 512, 8, 6, 2048, 0
B, S = 2, 2048
DK = D // H          # 64
P = 128
T = 512              # tokens per core
HT = T // 2          # tokens per half (tail pipeline granularity)
NJ = D // P          # 4   D-chunks
NM = DFF // P        # 16  DFF-chunks
NC = S // P          # 16  key chunks per batch row
NCORES = 8
GROUP = 4            # cores per batch group
EPS = 1e-5
SQRT_D = math.sqrt(D)
NEG = -80.0          # mask bias: exp(s-80) is zero for practical purposes

REPLICA_GROUPS = [[0, 1, 2, 3], [4, 5, 6, 7]]


def _patch_act_tables():
    """Steer every Exp activation to natural_log_exp_and_others (which holds
    both exp and ln) so the table never reloads between LayerNorm (Ln) and
    attention softmax (Exp).  Only set *contents* are changed; set order (and
    hence act_func_set ids) is preserved."""
    import concourse.hw_specs as hw_specs

    if getattr(hw_specs, "_exp_set_patch", False):
        return
    orig = hw_specs.get_activation_tables

    import functools

    @functools.cache
    def patched(module_arch):
        tabs = {k: set(v) for k, v in orig(module_arch).items()}
        if "natural_log_exp_and_others" in tabs:
            for name, fns in tabs.items():
                if name != "natural_log_exp_and_others":
                    fns.discard(AF.Exp)
        return tabs

    hw_specs.get_activation_tables = patched
    hw_specs._exp_set_patch = True
    bacc.get_activation_tables = patched


def build_kernel(use_b1, use_b2, use_ln1, use_ln2, use_fn):
    _patch_act_tables()
    nc = bacc.Bacc("TRN2", target_bir_lowering=False, debug=False,
                   num_devices=NCORES)

    # register EPS as a const AP so activation(bias=EPS) works
    _eps_t = nc.alloc_sbuf_tensor("const-float32-eps", [P, 1], F32)
    nc.gpsimd.memset(_eps_t.ap(), EPS)
    nc.const_aps.aps[(F32, EPS)] = _eps_t.ap()

    # ---------------- parameters ----------------
    tok = nc.declare_dram_parameter("tok", [P, NJ], I32, isOutput=False)
    emb = nc.declare_dram_parameter("emb", [VOCAB, D], F32, isOutput=False)
    pet = nc.declare_dram_parameter("pet", [P, NJ, T], F32, isOutput=False)
    wq = nc.declare_dram_parameter("wq", [L, P, NJ, D], BF16, isOutput=False)
    wk = nc.declare_dram_parameter("wk", [L, P, NJ, D], BF16, isOutput=False)
    wv = nc.declare_dram_parameter("wv", [L, P, NJ, D], BF16, isOutput=False)
    wo = nc.declare_dram_parameter("wo", [L, P, NJ, D], BF16, isOutput=False)
    w1 = nc.declare_dram_parameter("w1", [L, P, NJ, DFF], BF16, isOutput=False)
    w2 = nc.declare_dram_parameter("w2", [L, P, NM, D], BF16, isOutput=False)
    b1t = nc.declare_dram_parameter("b1t", [L, P, NM], F32, isOutput=False)
    b2t = nc.declare_dram_parameter("b2t", [L, P, NJ], F32, isOutput=False)
    lnsb = nc.declare_dram_parameter("lnsb", [P, 2 * L + 1, 2, NJ], F32,
                                     isOutput=False)
    kmaskn = nc.declare_dram_parameter("kmaskn", [P, NC], F32, isOutput=False)
    ones_bf = nc.declare_dram_parameter("ones_bf", [P, 1], BF16, isOutput=False)
    oneD = nc.declare_dram_parameter("oneD", [P, 1], F32, isOutput=False)
    ones_row = nc.declare_dram_parameter("ones_row", [1, P], F32, isOutput=False)
    sel2 = nc.declare_dram_parameter("sel2", [2, P], F32, isOutput=False)
    ident = nc.declare_dram_parameter("ident", [P, P], F32, isOutput=False)
    y = nc.declare_dram_parameter("y", [T, D], F32, isOutput=True)

    use_lnp = use_ln1 or use_ln2 or use_fn

    with tile.TileContext(nc) as tc:
        with (
            tc.tile_pool(name="wpool", bufs=2) as wpool,
            tc.tile_pool(name="work", bufs=1) as work,
            tc.tile_pool(name="small", bufs=2) as small,
            tc.tile_pool(name="kv", bufs=3) as kvp,
            tc.tile_pool(name="expp", bufs=3) as expp,
            tc.tile_pool(name="const", bufs=1) as constp,
            tc.tile_pool(name="ps_s", bufs=2, space="PSUM") as ps_s,
            tc.tile_pool(name="ps_av", bufs=2, space="PSUM") as ps_av,
            tc.tile_pool(name="ps_z", bufs=2, space="PSUM") as ps_z,
            tc.tile_pool(name="dram", bufs=2, space="DRAM") as dram,
        ):
            # ---------------- constants / persistent state ----------------
            tok_sb = constp.tile([P, NJ], I32)
            nc.sync.dma_start(tok_sb[:], tok[:])
            ones_bf_sb = constp.tile([P, 1], BF16)
            nc.sync.dma_start(ones_bf_sb[:], ones_bf[:])
            oneD_sb = constp.tile([P, 1], F32)
            nc.sync.dma_start(oneD_sb[:], oneD[:])
            ones_row_sb = constp.tile([1, P], F32)
            nc.sync.dma_start(ones_row_sb[:], ones_row[:])
            ident_sb = constp.tile([P, P], F32)
            nc.sync.dma_start(ident_sb[:], ident[:])
            pet_sb = constp.tile([P, NJ, T], F32)
            nc.sync.dma_start(pet_sb[:], pet[:])
            kmask_sb = constp.tile([P, NC], F32)
            nc.sync.dma_start(kmask_sb[:], kmaskn[:])
            lnp_sb = None
            if use_lnp:
                lnp_sb = constp.tile([P, 2 * L + 1, 2, NJ], F32, name="lnp")
                nc.sync.dma_start(lnp_sb[:], lnsb[:])

            xT = constp.tile([P, NJ, T], F32, name="xT")  # residual stream x.T

            def load_weights(l, what=("wq", "wk", "wv", "wo", "w1", "w2")):
                t = {}
                if "wq" in what:
                    t["wq"] = wpool.tile([P, NJ, D], BF16, tag="wq", name="wq")
                    nc.sync.dma_start(t["wq"][:], wq[l])
                if "wk" in what:
                    t["wk"] = wpool.tile([P, NJ, D], BF16, tag="wk", name="wk")
                    nc.sync.dma_start(t["wk"][:], wk[l])
                if "wv" in what:
                    t["wv"] = wpool.tile([P, NJ, D], BF16, tag="wv", name="wv")
                    nc.sync.dma_start(t["wv"][:], wv[l])
                if "wo" in what:
                    t["wo"] = wpool.tile([P, NJ, D], BF16, tag="wo", name="wo")
                    nc.sync.dma_start(t["wo"][:], wo[l])
                if "w1" in what:
                    t["w1"] = wpool.tile([P, NJ, DFF], BF16, tag="w1",
                                         name="w1")
                    nc.sync.dma_start(t["w1"][:], w1[l])
                    if use_b1:
                        t["b1"] = wpool.tile([P, NM], F32, tag="b1", name="b1")
                        nc.sync.dma_start(t["b1"][:], b1t[l])
                if "w2" in what:
                    t["w2"] = wpool.tile([P, NM, D], BF16, tag="w2", bufs=1,
                                         name="w2")
                    nc.sync.dma_start(t["w2"][:], w2[l])
                    if use_b2:
                        t["b2"] = wpool.tile([P, NJ], F32, tag="b2", name="b2")
                        nc.sync.dma_start(t["b2"][:], b2t[l])
                return t

            # ---------------- embedding gather + transpose ----------------
            for j in range(NJ):
                ex = small.tile([P, D], F32, tag="embx", bufs=1)
                nc.gpsimd.indirect_dma_start(
                    out=ex[:],
                    out_offset=None,
                    in_=emb[:, :],
                    in_offset=bass.IndirectOffsetOnAxis(ap=tok_sb[:, j:j + 1],
                                                        axis=0),
                )
                for dj in range(NJ):
                    pst = ps_s.tile([P, P], F32, tag="s")
                    nc.tensor.transpose(pst[:], ex[:, dj * P:(dj + 1) * P],
                                        ident_sb[:])
                    # x.T = emb.T * sqrt(D) + pe.T
                    sl = xT[:, dj, j * P:(j + 1) * P]
                    nc.vector.tensor_scalar(sl, pst[:], SQRT_D, None, OP.mult)
                    nc.vector.tensor_add(sl, sl, pet_sb[:, dj, j * P:(j + 1) * P])

            # ---------------- helpers ----------------
            def layernorm(out_t, param_idx, use_params, h=None):
                """LayerNorm over D (partition axis) of xT -> out_t [P,NJ,*].

                h=None: all T tokens; h=0/1: token half (HT columns)."""
                if h is None:
                    tsl = slice(0, T)
                    n = T
                else:
                    tsl = slice(h * HT, (h + 1) * HT)
                    n = HT
                sq = work.tile([P, NJ, n], F32, tag="lnt", name="sq")
                nc.vector.tensor_tensor(sq[:], xT[:, :, tsl], xT[:, :, tsl],
                                        OP.mult)
                st = ps_s.tile([P, n], F32, tag="s", name="st")
                for j in range(NJ):
                    nc.tensor.matmul(st[0:1, :], lhsT=oneD_sb[:],
                                     rhs=xT[:, j, tsl],
                                     start=(j == 0), stop=(j == NJ - 1),
                                     tile_position=(0, 0))
                    nc.tensor.matmul(st[32:33, :], lhsT=oneD_sb[:],
                                     rhs=sq[:, j, :],
                                     start=(j == 0), stop=(j == NJ - 1),
                                     tile_position=(0, 32))
                mu_sb = small.tile([1, n], F32, tag="mu_sb", bufs=1)
                nc.vector.tensor_copy(mu_sb[:], st[0:1, :])
                ex2_sb = small.tile([1, n], F32, tag="ex2_sb", bufs=1)
                nc.vector.tensor_copy(ex2_sb[:], st[32:33, :])
                mu = mu_sb[:]
                var = small.tile([1, n], F32, tag="var", bufs=1)
                nc.vector.tensor_tensor(var[:], mu, mu, OP.mult)
                nc.vector.tensor_tensor(var[:], ex2_sb[:], var[:], OP.subtract)
                lnv = small.tile([1, n], F32, tag="lnv", bufs=1)
                nc.scalar.activation(lnv[:], var[:], AF.Ln, bias=EPS)
                rinv = small.tile([1, n], F32, tag="rinv", bufs=1)
                nc.scalar.activation(rinv[:], lnv[:], AF.Exp, scale=-0.5)
                bc_mu = ps_s.tile([P, n], F32, tag="s", name="bc_mu")
                nc.tensor.matmul(bc_mu[:], lhsT=ones_row_sb[:], rhs=mu,
                                 start=True, stop=True, tile_position=(0, 0))
                bc_ri = ps_s.tile([P, n], F32, tag="s", name="bc_ri")
                nc.tensor.matmul(bc_ri[:], lhsT=ones_row_sb[:], rhs=rinv[:],
                                 start=True, stop=True, tile_position=(0, 0))
                t1 = work.tile([P, NJ, n], F32, tag="lnt", name="lnt")
                nc.vector.tensor_tensor(
                    t1[:], xT[:, :, tsl],
                    bc_mu[:, None, :].to_broadcast([P, NJ, n]),
                    OP.subtract)
                nc.vector.tensor_tensor(
                    out_t[:], t1[:], bc_ri[:, None, :].to_broadcast([P, NJ, n]),
                    OP.mult)
                if use_params:
                    for j in range(NJ):
                        nc.vector.tensor_scalar(
                            out_t[:, j, :], out_t[:, j, :],
                            lnp_sb[:, param_idx, 0, j:j + 1],
                            lnp_sb[:, param_idx, 1, j:j + 1],
                            OP.mult, OP.add)

            # ---- tail-half building blocks (token half h of tokens tsl) ----
            def wo_half(w_sb, attnT, h):
                tsl = slice(h * HT, (h + 1) * HT)
                for m in range(NJ):
                    ps = ps_s.tile([P, HT], F32, tag="s", name="omm")
                    for j in range(NJ):
                        nc.tensor.matmul(
                            ps[:], lhsT=w_sb["wo"][:, j, m * P:(m + 1) * P],
                            rhs=attnT[:, j, tsl],
                            start=(j == 0), stop=(j == NJ - 1),
                            tile_position=(0, 0))
                    nc.vector.tensor_add(xT[:, m, tsl], xT[:, m, tsl], ps[:])

            def ffn_half(w_sb, gT, h):
                tsl = slice(h * HT, (h + 1) * HT)
                h1T = work.tile([P, NM, HT], BF16, tag="h1T", name="h1T")
                for m in range(NM):
                    ps = ps_s.tile([P, HT], F32, tag="s", name="f1mm")
                    for j in range(NJ):
                        nc.tensor.matmul(
                            ps[:], lhsT=w_sb["w1"][:, j, m * P:(m + 1) * P],
                            rhs=gT[:, j, :],
                            start=(j == 0), stop=(j == NJ - 1),
                            tile_position=(0, 0))
                    if use_b1:
                        nc.vector.tensor_scalar(h1T[:, m, :], ps[:],
                                                w_sb["b1"][:, m:m + 1], 0.0,
                                                OP.add, OP.max)
                    else:
                        nc.vector.tensor_scalar(h1T[:, m, :], ps[:], 0.0, None,
                                                OP.max)
                for m in range(NJ):
                    ps = ps_s.tile([P, HT], F32, tag="s", name="f2mm")
                    for j in range(NM):
                        nc.tensor.matmul(
                            ps[:], lhsT=w_sb["w2"][:, j, m * P:(m + 1) * P],
                            rhs=h1T[:, j, :],
                            start=(j == 0), stop=(j == NM - 1),
                            tile_position=(0, 0))
                    if use_b2:
                        tmp = small.tile([P, HT], F32, tag="b2tmp",
                                         name="b2tmp")
                        nc.vector.tensor_scalar(tmp[:], ps[:],
                                                w_sb["b2"][:, m:m + 1], None,
                                                OP.add)
                        nc.vector.tensor_add(xT[:, m, tsl], xT[:, m, tsl],
                                             tmp[:])
                    else:
                        nc.vector.tensor_add(xT[:, m, tsl], xT[:, m, tsl],
                                             ps[:])

            def kv_half(w_sb, hT, kT, vloc, h):
                """K^T chunks (all heads, token half) + V (token chunks of h)."""
                tsl = slice(h * HT, (h + 1) * HT)
                for m in range(NJ):
                    ps = ps_s.tile([P, HT], F32, tag="s", name="kmm")
                    for j in range(NJ):
                        nc.tensor.matmul(
                            ps[:], lhsT=w_sb["wk"][:, j, m * P:(m + 1) * P],
                            rhs=hT[:, j, tsl],
                            start=(j == 0), stop=(j == NJ - 1),
                            tile_position=(0, 0))
                    nc.vector.tensor_copy(kT[:, m, tsl], ps[:])
                for mt in (2 * h, 2 * h + 1):
                    ps = ps_s.tile([P, D], F32, tag="s", name="vmm")
                    for j in range(NJ):
                        nc.tensor.matmul(
                            ps[:], lhsT=hT[:, j, mt * P:(mt + 1) * P],
                            rhs=w_sb["wv"][:, j, :],
                            start=(j == 0), stop=(j == NJ - 1),
                            tile_position=(0, 0))
                    nc.vector.tensor_copy(vloc[:, mt, :], ps[:])

            def q_half(w_sb, hT, qT, h):
                tsl = slice(h * HT, (h + 1) * HT)
                for m in range(NJ):
                    ps = ps_s.tile([P, HT], F32, tag="s", name="qmm")
                    for j in range(NJ):
                        nc.tensor.matmul(
                            ps[:], lhsT=w_sb["wq"][:, j, m * P:(m + 1) * P],
                            rhs=hT[:, j, tsl],
                            start=(j == 0), stop=(j == NJ - 1),
                            tile_position=(0, 0))
                    nc.vector.tensor_scalar(qT[:, m, tsl], ps[:],
                                            1.0 / math.sqrt(DK), None, OP.mult)

            def launch_ag(kT, vloc, h, l):
                """AG half h: K^T (all heads) + V for token half h."""
                tsl = slice(h * HT, (h + 1) * HT)
                kv_h = dram.tile([2, NJ * P * HT], BF16, tag=f"kv{h}",
                                 name=f"kv{h}_{l}")
                nc.sync.dma_start(
                    kv_h[0].rearrange("(j p t) -> p j t", p=P, j=NJ),
                    kT[:, :, tsl])
                nc.sync.dma_start(
                    kv_h[1].rearrange("(m p d) -> p m d", p=P, m=2),
                    vloc[:, 2 * h:2 * h + 2, :])
                ag_h = dram.tile([GROUP, 2, NJ * P * HT], BF16, tag=f"ag{h}",
                                 name=f"ag{h}_{l}")
                nc.gpsimd.collective_compute(
                    "AllGather", OP.bypass, replica_groups=REPLICA_GROUPS,
                    ins=[kv_h[:].opt()], outs=[ag_h[:].opt()],
                )
                return ag_h

            def attention(qT, ags, attnT):
                """Per head pair: scores/exp/AV over 16 key chunks, AG-half-0
                key chunks first.  Each pair's 1/Z normalize is deferred into
                the next pair's first chunk so the av/z PSUM slots never
                head-of-line block the tensor engine."""
                pending = []   # [(av, z, pair)] awaiting normalize

                def normalize():
                    av, z, pair = pending.pop(0)
                    rinv0 = small.tile([1, T], F32, tag="zrec0", bufs=1,
                                       name="zrec0")
                    nc.vector.reciprocal_approx_fast(rinv0[:], z[0:1, :])
                    rinv1 = small.tile([1, T], F32, tag="zrec1", bufs=1,
                                       name="zrec1")
                    nc.vector.reciprocal_approx_fast(rinv1[:], z[32:33, :])
                    bc = ps_z.tile([P, T], F32, tag="z", name="recbc")
                    nc.tensor.matmul(bc[0:DK, :], lhsT=ones_row_sb[0:1, 0:DK],
                                     rhs=rinv0[:], start=True, stop=True,
                                     tile_position=(0, 0))
                    nc.tensor.matmul(bc[DK:P, :], lhsT=ones_row_sb[0:1, 0:DK],
                                     rhs=rinv1[:], start=True, stop=True,
                                     tile_position=(0, 64))
                    bc_sb = small.tile([P, T], F32, tag="bc_sb", bufs=1,
                                       name="bc_sb")
                    nc.vector.tensor_copy(bc_sb[:], bc[:])
                    nc.vector.tensor_tensor(attnT[:, pair, :], av[:],
                                            bc_sb[:], OP.mult)

                for pair in range(H // 2):
                    kTp = kvp.tile([P, S], BF16, tag="kTp", bufs=3,
                                   name=f"kTp{pair}")
                    vps = []
                    for h in range(2):
                        for r in range(GROUP):
                            nc.sync.dma_start(
                                kTp[:, r * T + h * HT:r * T + (h + 1) * HT],
                                ags[h][r, 0]
                                .rearrange("(j p t) -> p j t", p=P, j=NJ)
                                [:, pair, :])
                        vp = kvp.tile([P, 2 * GROUP, P], BF16, tag=f"vp{h}",
                                      bufs=3, name=f"vp{h}_{pair}")
                        for r in range(GROUP):
                            nc.sync.dma_start(
                                vp[:, 2 * r:2 * r + 2, :],
                                ags[h][r, 1]
                                .rearrange("(m p d) -> p m d", p=P, m=2)
                                [:, :, pair * P:(pair + 1) * P])
                        vps.append(vp)

                    av = ps_av.tile([P, T], F32, tag="av", name="av")
                    z = ps_z.tile([P, T], F32, tag="z", name="z")
                    # all AG-half-0 key chunks first, then half 1
                    order = [(r, 2 * h + j) for h in range(2)
                             for r in range(GROUP) for j in range(2)]
                    for ci, (r, cl) in enumerate(order):
                        c = r * NJ + cl            # global key chunk
                        h, j = cl // 2, cl % 2
                        pss = ps_s.tile([P, 2, T], F32, tag="s", name="pss")
                        nc.tensor.matmul(pss[:, 0, :],
                                         lhsT=kTp[0:DK, c * P:(c + 1) * P],
                                         rhs=qT[0:DK, pair, :],
                                         start=True, stop=True,
                                         tile_position=(0, 0))
                        nc.tensor.matmul(pss[:, 1, :],
                                         lhsT=kTp[DK:P, c * P:(c + 1) * P],
                                         rhs=qT[DK:P, pair, :],
                                         start=True, stop=True,
                                         tile_position=(64, 0))
                        ep = expp.tile([P, 2, T], BF16, tag="ep", name="ep")
                        nc.scalar.activation(ep[:], pss[:], AF.Exp,
                                             bias=kmask_sb[:, c:c + 1])
                        if ci == 0 and pending:
                            normalize()   # previous pair, overlaps this exp
                        first, last = (ci == 0), (ci == NC - 1)
                        vtile = vps[h][:, 2 * r + j, :]
                        nc.tensor.matmul(av[0:DK, :], lhsT=vtile[:, 0:DK],
                                         rhs=ep[:, 0, :], start=first,
                                         stop=last, tile_position=(0, 0),
                                         skip_group_check=True)
                        nc.tensor.matmul(av[DK:P, :], lhsT=vtile[:, DK:P],
                                         rhs=ep[:, 1, :], start=first,
                                         stop=last, tile_position=(0, 64),
                                         skip_group_check=True)
                        nc.tensor.matmul(z[0:1, :], lhsT=ones_bf_sb[:],
                                         rhs=ep[:, 0, :], start=first,
                                         stop=last, tile_position=(0, 0),
                                         skip_group_check=True)
                        nc.tensor.matmul(z[32:33, :], lhsT=ones_bf_sb[:],
                                         rhs=ep[:, 1, :], start=first,
                                         stop=last, tile_position=(0, 32),
                                         skip_group_check=True)
                    pending.append((av, z, pair))
                while pending:
                    normalize()

            # ---------------- prologue: LN1(0) + KV(0)/AG(0)/Q(0) ----------
            w_sb = load_weights(0)
            hT = work.tile([P, NJ, T], BF16, tag="hT", name="hT")
            kT = work.tile([P, NJ, T], BF16, tag="kT", name="kT")
            vloc = work.tile([P, NJ, D], BF16, tag="vloc", name="vloc")
            qT = work.tile([P, NJ, T], BF16, tag="qT", name="qT")
            ags = []
            for h in range(2):
                layernorm(hT[:, :, h * HT:(h + 1) * HT], 0, use_ln1, h)
                kv_half(w_sb, hT, kT, vloc, h)
                ags.append(launch_ag(kT, vloc, h, 0))
                q_half(w_sb, hT, qT, h)

            # ---------------- layers ----------------
            for l in range(L):
                attnT = work.tile([P, NJ, T], BF16, tag="attnT", name="attnT")
                attention(qT, ags, attnT)

                if l + 1 < L:
                    w_next = load_weights(l + 1, what=("wq", "wk", "wv"))
                    hT = work.tile([P, NJ, T], BF16, tag="hT", name="hT")
                    kT = work.tile([P, NJ, T], BF16, tag="kT", name="kT")
                    vloc = work.tile([P, NJ, D], BF16, tag="vloc",
                                     name="vloc")
                    qT = work.tile([P, NJ, T], BF16, tag="qT", name="qT")
                    ags = []

                for h in range(2):
                    wo_half(w_sb, attnT, h)
                    gT = work.tile([P, NJ, HT], BF16, tag="gT", name="gT")
                    layernorm(gT, 2 * l + 1, use_ln2, h)
                    ffn_half(w_sb, gT, h)
                    if l + 1 < L:
                        layernorm(hT[:, :, h * HT:(h + 1) * HT],
                                  2 * (l + 1), use_ln1, h)
                        kv_half(w_next, hT, kT, vloc, h)
                        ags.append(launch_ag(kT, vloc, h, l + 1))
                        q_half(w_next, hT, qT, h)
                        if h == 1:
                            w_next.update(load_weights(
                                l + 1, what=("wo", "w1", "w2")))

                if l + 1 < L:
                    w_sb = w_next

            # ---------------- final LN + output ----------------
            outT = work.tile([P, NJ, T], F32, tag="outT", name="outT")
            layernorm(outT, 2 * L, use_fn)
            out_sb = work.tile([P, NJ, D], F32, tag="lnt", name="out_sb")
            for dj in range(NJ):
                for tj in range(NJ):
                    pst = ps_s.tile([P, P], F32, tag="s", name="otr")
                    nc.tensor.transpose(pst[:], outT[:, dj, tj * P:(tj + 1) * P],
                                        ident_sb[:])
                    nc.vector.tensor_copy(out_sb[:, tj, dj * P:(dj + 1) * P],
                                          pst[:])
            nc.sync.dma_start(y.rearrange("(j p) d -> p j d", p=P), out_sb[:])
        import sys, time
        print(f"[build] body traced {time.time():.0f}", file=sys.stderr, flush=True)

    print(f"[build] tile scheduled {time.time():.0f}", file=sys.stderr, flush=True)
    nc.compile()
    print(f"[build] bacc compiled {time.time():.0f}", file=sys.stderr, flush=True)
    return nc


# ---------------------------------------------------------------------------
_CACHE = {}


def _get_kernel(flags):
    if flags not in _CACHE:
        _CACHE[flags] = build_kernel(*flags)
    return _CACHE[flags]


def _chunkP(a):
    """[..., n*P, m] -> [..., P, n, m] with dim = n_idx*P + p."""
    a = np.asarray(a)
    *lead, npm, m = a.shape
    n = npm // P
    return np.ascontiguousarray(a.reshape(*lead, n, P, m).swapaxes(-3, -2))


def kernel(**inputs):
    src = np.asarray(inputs["src"]).astype(np.int64)
    emb = np.asarray(inputs["emb"], np.float32)
    pe = np.asarray(inputs["pe"], np.float32)
    W = {k: np.asarray(inputs[k], np.float32)
         for k in ("WQ", "WK", "WV", "WO", "W1", "W2", "b1", "b2",
                   "ln1_s", "ln1_b", "ln2_s", "ln2_b", "fn_s", "fn_b")}

    use_b1 = bool(np.any(W["b1"] != 0.0))
    use_b2 = bool(np.any(W["b2"] != 0.0))
    use_ln1 = bool(np.any(W["ln1_s"] != 1.0) or np.any(W["ln1_b"] != 0.0))
    use_ln2 = bool(np.any(W["ln2_s"] != 1.0) or np.any(W["ln2_b"] != 0.0))
    use_fn = bool(np.any(W["fn_s"] != 1.0) or np.any(W["fn_b"] != 0.0))
    nc = _get_kernel((use_b1, use_b2, use_ln1, use_ln2, use_fn))

    def perD(a):  # [L, D] -> [L, P, NJ] (d = j*P + p)
        a = np.asarray(a, np.float32)
        return np.ascontiguousarray(a.reshape(-1, NJ, P).swapaxes(-2, -1))

    lnsb = np.zeros((P, 2 * L + 1, 2, NJ), np.float32)
    for l in range(L):
        lnsb[:, 2 * l, 0] = perD(W["ln1_s"])[l]
        lnsb[:, 2 * l, 1] = perD(W["ln1_b"])[l]
        lnsb[:, 2 * l + 1, 0] = perD(W["ln2_s"])[l]
        lnsb[:, 2 * l + 1, 1] = perD(W["ln2_b"])[l]
    lnsb[:, 2 * L, 0] = perD(W["fn_s"][None])[0]
    lnsb[:, 2 * L, 1] = perD(W["fn_b"][None])[0]

    sel = np.zeros((2, P), np.float32)
    sel[0, :DK] = 1.0
    sel[1, DK:] = 1.0

    shared = {
        "emb": emb,
        "wq": _chunkP(W["WQ"]).astype(ml_dtypes.bfloat16),
        "wk": _chunkP(W["WK"]).astype(ml_dtypes.bfloat16),
        "wv": _chunkP(W["WV"]).astype(ml_dtypes.bfloat16),
        "wo": _chunkP(W["WO"]).astype(ml_dtypes.bfloat16),
        "w1": _chunkP(W["W1"]).astype(ml_dtypes.bfloat16),
        "w2": _chunkP(W["W2"]).astype(ml_dtypes.bfloat16),
        "b1t": np.ascontiguousarray(W["b1"].reshape(L, NM, P).swapaxes(1, 2)),
        "b2t": np.ascontiguousarray(W["b2"].reshape(L, NJ, P).swapaxes(1, 2)),
        "lnsb": lnsb,
        "ones_bf": np.ones((P, 1), ml_dtypes.bfloat16),
        "oneD": np.full((P, 1), 1.0 / D, np.float32),
        "ones_row": np.ones((1, P), np.float32),
        "sel2": sel,
        "ident": np.eye(P, dtype=np.float32),
    }

    in_maps = []
    for c in range(NCORES):
        b, sc = c // GROUP, c % GROUP
        ids = src[b, sc * T:(sc + 1) * T].astype(np.int32)
        peslice = pe[sc * T:(sc + 1) * T, :]                   # [T, D]
        pett = np.ascontiguousarray(
            peslice.T.reshape(NJ, P, T).swapaxes(0, 1))        # [P, NJ, T]
        maskn = np.where(src[b] == PAD, NEG, 0.0).astype(np.float32)
        m = dict(shared)
        m["tok"] = np.ascontiguousarray(ids.reshape(NJ, P).T)
        m["pet"] = pett.astype(np.float32)
        m["kmaskn"] = np.ascontiguousarray(maskn.reshape(NC, P).T)
        in_maps.append(m)

    res = run_bass_kernel_spmd(nc, in_maps, core_ids=list(range(NCORES)))
    out = np.empty((B, S, D), np.float32)
    for c in range(NCORES):
        b, sc = c // GROUP, c % GROUP
        out[b, sc * T:(sc + 1) * T, :] = res.results[c]["y"]
    return out


# revision 19
# speedup vs baseline: 1.0937x; 1.0937x over previous
"""Trainium2 Bass kernel for a 6-layer pre-LN transformer encoder (nn_Encoder).

Distribution: tokens sharded 8 ways (core c -> batch c//4, seq chunk c%4 of 512
tokens).  Per layer, each core computes K^T/V projections for its own tokens,
AllGathers them (bf16) within its batch group of 4 cores, and computes
attention for its 512 query tokens over the full 2048-key sequence.
Projections / FFN / LayerNorms are purely local to the token shard.

The residual stream is kept transposed (x.T: [D, tok]) so every matmul
consumes operands in natural layout with no per-layer transposes:
  q.T/k.T = W.T @ h.T    (lhsT = W chunk,   rhs = h.T chunk)
  v       = h @ W        (lhsT = h.T chunk, rhs = W chunk)
  scores.T[k,q]          (lhsT = K.T chunk, rhs = q.T head)  [2 heads row-packed]
  attn.T  = V.T @ P.T    (lhsT = V chunk,   rhs = expP)      [2 heads col-packed]
  Z       = ones.T @ P.T                                     [2 heads col-packed]
  out.T   = WO.T @ attn.T; FFN analogous.
LayerNorm reductions over D (partition axis) use (1/D)-valued ones-matmuls;
per-token scalars are broadcast across partitions with K=1 ones matmuls.
rsqrt(var+eps) = Exp(-0.5 * Ln(var+eps)) keeps ACT on one table set.
The key-padding mask folds into the exp() activation bias (per-partition).

v2 pipeline: everything after attention (WO/LN2/FFN/LN1'/KV'/Q') is processed
in two token halves; the K/V AllGather for layer l+1 is split by token half
and launched at the end of each half's tail work, so the collective overlaps
the other half's FFN + the next attention's first chunks.  Softmax 1/Z uses a
single custom-DVE reciprocal_approx_fast over both packed head rows.
"""

import math

import numpy as np
import ml_dtypes

import concourse.bass as bass
import concourse.mybir as mybir
import concourse.tile as tile
from concourse import bacc
from concourse.bass_utils import run_bass_kernel_spmd

F32 = mybir.dt.float32
BF16 = mybir.dt.bfloat16
I32 = mybir.dt.int32
AF = mybir.ActivationFunctionType
OP = mybir.AluOpType

VOCAB, D, H, L, DFF, PAD = 32000, 512, 8, 6, 2048, 0
B, S = 2, 2048
DK = D // H          # 64
P = 128
T = 512              # tokens per core
HT = T // 2          # tokens per half (tail pipeline granularity)
NJ = D // P          # 4   D-chunks
NM = DFF // P        # 16  DFF-chunks
NC = S // P          # 16  key chunks per batch row
NCORES = 8
GROUP = 4            # cores per batch group
EPS = 1e-5
SQRT_D = math.sqrt(D)
NEG = -80.0          # mask bias: exp(s-80) is zero for practical purposes

REPLICA_GROUPS = [[0, 1, 2, 3], [4, 5, 6, 7]]


def _patch_act_tables():
    """Steer every Exp activation to natural_log_exp_and_others (which holds
    both exp and ln) so the table never reloads between LayerNorm (Ln) and
    attention softmax (Exp).  Only set *contents* are changed; set order (and
    hence act_func_set ids) is preserved."""
    import concourse.hw_specs as hw_specs

    if getattr(hw_specs, "_exp_set_patch", False):
        return
    orig = hw_specs.get_activation_tables

    import functools

    @functools.cache
    def patched(module_arch):
        tabs = {k: set(v) for k, v in orig(module_arch).items()}
        if "natural_log_exp_and_others" in tabs:
            for name, fns in tabs.items():
                if name != "natural_log_exp_and_others":
                    fns.discard(AF.Exp)
        return tabs

    hw_specs.get_activation_tables = patched
    hw_specs._exp_set_patch = True
    bacc.get_activation_tables = patched


def build_kernel(use_b1, use_b2, use_ln1, use_ln2, use_fn):
    _patch_act_tables()
    nc = bacc.Bacc("TRN2", target_bir_lowering=False, debug=False,
                   num_devices=NCORES)

    # register EPS as a const AP so activation(bias=EPS) works
    _eps_t = nc.alloc_sbuf_tensor("const-float32-eps", [P, 1], F32)
    nc.gpsimd.memset(_eps_t.ap(), EPS)
    nc.const_aps.aps[(F32, EPS)] = _eps_t.ap()

    # ---------------- parameters ----------------
    tok = nc.declare_dram_parameter("tok", [P, NJ], I32, isOutput=False)
    emb = nc.declare_dram_parameter("emb", [VOCAB, D], F32, isOutput=False)
    pet = nc.declare_dram_parameter("pet", [P, NJ, T], F32, isOutput=False)
    wq = nc.declare_dram_parameter("wq", [L, P, NJ, D], BF16, isOutput=False)
    wk = nc.declare_dram_parameter("wk", [L, P, NJ, D], BF16, isOutput=False)
    wv = nc.declare_dram_parameter("wv", [L, P, NJ, D], BF16, isOutput=False)
    wo = nc.declare_dram_parameter("wo", [L, P, NJ, D], BF16, isOutput=False)
    w1 = nc.declare_dram_parameter("w1", [L, P, NJ, DFF], BF16, isOutput=False)
    w2 = nc.declare_dram_parameter("w2", [L, P, NM, D], BF16, isOutput=False)
    b1t = nc.declare_dram_parameter("b1t", [L, P, NM], F32, isOutput=False)
    b2t = nc.declare_dram_parameter("b2t", [L, P, NJ], F32, isOutput=False)
    lnsb = nc.declare_dram_parameter("lnsb", [P, 2 * L + 1, 2, NJ], F32,
                                     isOutput=False)
    kmaskn = nc.declare_dram_parameter("kmaskn", [P, NC], F32, isOutput=False)
    ones_bf = nc.declare_dram_parameter("ones_bf", [P, 1], BF16, isOutput=False)
    oneD = nc.declare_dram_parameter("oneD", [P, 1], F32, isOutput=False)
    ones_row = nc.declare_dram_parameter("ones_row", [1, P], F32, isOutput=False)
    sel2 = nc.declare_dram_parameter("sel2", [2, P], F32, isOutput=False)
    ident = nc.declare_dram_parameter("ident", [P, P], F32, isOutput=False)
    y = nc.declare_dram_parameter("y", [T, D], F32, isOutput=True)

    use_lnp = use_ln1 or use_ln2 or use_fn

    with tile.TileContext(nc) as tc:
        with (
            tc.tile_pool(name="wpool", bufs=2) as wpool,
            tc.tile_pool(name="work", bufs=1) as work,
            tc.tile_pool(name="small", bufs=2) as small,
            tc.tile_pool(name="kv", bufs=3) as kvp,
            tc.tile_pool(name="expp", bufs=3) as expp,
            tc.tile_pool(name="const", bufs=1) as constp,
            tc.tile_pool(name="ps_s", bufs=2, space="PSUM") as ps_s,
            tc.tile_pool(name="ps_av", bufs=2, space="PSUM") as ps_av,
            tc.tile_pool(name="ps_z", bufs=2, space="PSUM") as ps_z,
            tc.tile_pool(name="dram", bufs=2, space="DRAM") as dram,
        ):
            # ---------------- constants / persistent state ----------------
            ones_bf_sb = constp.tile([P, 1], BF16)
            nc.sync.dma_start(ones_bf_sb[:], ones_bf[:])
            oneD_sb = constp.tile([P, 1], F32)
            nc.sync.dma_start(oneD_sb[:], oneD[:])
            ones_row_sb = constp.tile([1, P], F32)
            nc.sync.dma_start(ones_row_sb[:], ones_row[:])
            ident_sb = constp.tile([P, P], F32)
            nc.sync.dma_start(ident_sb[:], ident[:])
            pet_sb = constp.tile([P, NJ, T], F32)
            nc.sync.dma_start(pet_sb[:], pet[:])
            kmask_sb = constp.tile([P, NC], F32)
            nc.sync.dma_start(kmask_sb[:], kmaskn[:])
            tok_sb = constp.tile([P, NJ], I32)
            nc.sync.dma_start(tok_sb[:], tok[:])
            lnp_sb = None
            if use_lnp:
                lnp_sb = constp.tile([P, 2 * L + 1, 2, NJ], F32, name="lnp")
                nc.sync.dma_start(lnp_sb[:], lnsb[:])

            xT = constp.tile([P, NJ, T], F32, name="xT")  # residual stream x.T

            def load_weights(l, what=("wq", "wk", "wv", "wo", "w1", "w2")):
                t = {}
                if "wq" in what:
                    t["wq"] = wpool.tile([P, NJ, D], BF16, tag="wq", name="wq")
                    nc.sync.dma_start(t["wq"][:], wq[l])
                if "wk" in what:
                    t["wk"] = wpool.tile([P, NJ, D], BF16, tag="wk", name="wk")
                    nc.sync.dma_start(t["wk"][:], wk[l])
                if "wv" in what:
                    t["wv"] = wpool.tile([P, NJ, D], BF16, tag="wv", name="wv")
                    nc.sync.dma_start(t["wv"][:], wv[l])
                if "wo" in what:
                    t["wo"] = wpool.tile([P, NJ, D], BF16, tag="wo", name="wo")
                    nc.sync.dma_start(t["wo"][:], wo[l])
                if "w1" in what:
                    t["w1"] = wpool.tile([P, NJ, DFF], BF16, tag="w1",
                                         name="w1")
                    nc.sync.dma_start(t["w1"][:], w1[l])
                    if use_b1:
                        t["b1"] = wpool.tile([P, NM], F32, tag="b1", name="b1")
                        nc.sync.dma_start(t["b1"][:], b1t[l])
                if "w2" in what:
                    t["w2"] = wpool.tile([P, NM, D], BF16, tag="w2", bufs=1,
                                         name="w2")
                    nc.sync.dma_start(t["w2"][:], w2[l])
                    if use_b2:
                        t["b2"] = wpool.tile([P, NJ], F32, tag="b2", name="b2")
                        nc.sync.dma_start(t["b2"][:], b2t[l])
                return t

            # ---------------- embedding gather + transpose ----------------
            for j in range(NJ):
                ex = small.tile([P, D], F32, tag="embx", bufs=1)
                nc.gpsimd.indirect_dma_start(
                    out=ex[:],
                    out_offset=None,
                    in_=emb[:, :],
                    in_offset=bass.IndirectOffsetOnAxis(ap=tok_sb[:, j:j + 1],
                                                        axis=0),
                )
                for dj in range(NJ):
                    pst = ps_s.tile([P, P], F32, tag="s")
                    nc.tensor.transpose(pst[:], ex[:, dj * P:(dj + 1) * P],
                                        ident_sb[:])
                    # x.T = emb.T * sqrt(D) + pe.T
                    sl = xT[:, dj, j * P:(j + 1) * P]
                    nc.vector.tensor_scalar(sl, pst[:], SQRT_D, None, OP.mult)
                    nc.vector.tensor_add(sl, sl, pet_sb[:, dj, j * P:(j + 1) * P])

            # ---------------- helpers ----------------
            def layernorm(out_t, param_idx, use_params, h=None):
                """LayerNorm over D (partition axis) of xT -> out_t [P,NJ,*].

                h=None: all T tokens; h=0/1: token half (HT columns)."""
                if h is None:
                    tsl = slice(0, T)
                    n = T
                else:
                    tsl = slice(h * HT, (h + 1) * HT)
                    n = HT
                sq = work.tile([P, NJ, n], F32, tag="lnt", name="sq")
                nc.vector.tensor_tensor(sq[:], xT[:, :, tsl], xT[:, :, tsl],
                                        OP.mult)
                st = ps_s.tile([P, n], F32, tag="s", name="st")
                for j in range(NJ):
                    nc.tensor.matmul(st[0:1, :], lhsT=oneD_sb[:],
                                     rhs=xT[:, j, tsl],
                                     start=(j == 0), stop=(j == NJ - 1),
                                     tile_position=(0, 0))
                    nc.tensor.matmul(st[32:33, :], lhsT=oneD_sb[:],
                                     rhs=sq[:, j, :],
                                     start=(j == 0), stop=(j == NJ - 1),
                                     tile_position=(0, 32))
                mu_sb = small.tile([1, n], F32, tag="mu_sb", bufs=1)
                nc.vector.tensor_copy(mu_sb[:], st[0:1, :])
                ex2_sb = small.tile([1, n], F32, tag="ex2_sb", bufs=1)
                nc.vector.tensor_copy(ex2_sb[:], st[32:33, :])
                mu = mu_sb[:]
                var = small.tile([1, n], F32, tag="var", bufs=1)
                nc.vector.tensor_tensor(var[:], mu, mu, OP.mult)
                nc.vector.tensor_tensor(var[:], ex2_sb[:], var[:], OP.subtract)
                lnv = small.tile([1, n], F32, tag="lnv", bufs=1)
                nc.scalar.activation(lnv[:], var[:], AF.Ln, bias=EPS)
                rinv = small.tile([1, n], F32, tag="rinv", bufs=1)
                nc.scalar.activation(rinv[:], lnv[:], AF.Exp, scale=-0.5)
                bc_mu = ps_s.tile([P, n], F32, tag="s", name="bc_mu")
                nc.tensor.matmul(bc_mu[:], lhsT=ones_row_sb[:], rhs=mu,
                                 start=True, stop=True, tile_position=(0, 0))
                bc_ri = ps_s.tile([P, n], F32, tag="s", name="bc_ri")
                nc.tensor.matmul(bc_ri[:], lhsT=ones_row_sb[:], rhs=rinv[:],
                                 start=True, stop=True, tile_position=(0, 0))
                t1 = work.tile([P, NJ, n], F32, tag="lnt", name="lnt")
                nc.vector.tensor_tensor(
                    t1[:], xT[:, :, tsl],
                    bc_mu[:, None, :].to_broadcast([P, NJ, n]),
                    OP.subtract)
                nc.vector.tensor_tensor(
                    out_t[:], t1[:], bc_ri[:, None, :].to_broadcast([P, NJ, n]),
                    OP.mult)
                if use_params:
                    for j in range(NJ):
                        nc.vector.tensor_scalar(
                            out_t[:, j, :], out_t[:, j, :],
                            lnp_sb[:, param_idx, 0, j:j + 1],
                            lnp_sb[:, param_idx, 1, j:j + 1],
                            OP.mult, OP.add)

            # ---- tail-half building blocks (token half h of tokens tsl) ----
            def wo_half(w_sb, attnT, h):
                tsl = slice(h * HT, (h + 1) * HT)
                for m in range(NJ):
                    ps = ps_s.tile([P, HT], F32, tag="s", name="omm")
                    for j in range(NJ):
                        nc.tensor.matmul(
                            ps[:], lhsT=w_sb["wo"][:, j, m * P:(m + 1) * P],
                            rhs=attnT[:, j, tsl],
                            start=(j == 0), stop=(j == NJ - 1),
                            tile_position=(0, 0))
                    nc.vector.tensor_add(xT[:, m, tsl], xT[:, m, tsl], ps[:])

            def ffn_half(w_sb, gT, h):
                tsl = slice(h * HT, (h + 1) * HT)
                h1T = work.tile([P, NM, HT], BF16, tag="h1T", name="h1T")
                for m in range(NM):
                    ps = ps_s.tile([P, HT], F32, tag="s", name="f1mm")
                    for j in range(NJ):
                        nc.tensor.matmul(
                            ps[:], lhsT=w_sb["w1"][:, j, m * P:(m + 1) * P],
                            rhs=gT[:, j, :],
                            start=(j == 0), stop=(j == NJ - 1),
                            tile_position=(0, 0))
                    if use_b1:
                        nc.vector.tensor_scalar(h1T[:, m, :], ps[:],
                                                w_sb["b1"][:, m:m + 1], 0.0,
                                                OP.add, OP.max)
                    else:
                        nc.vector.tensor_scalar(h1T[:, m, :], ps[:], 0.0, None,
                                                OP.max)
                for m in range(NJ):
                    ps = ps_s.tile([P, HT], F32, tag="s", name="f2mm")
                    for j in range(NM):
                        nc.tensor.matmul(
                            ps[:], lhsT=w_sb["w2"][:, j, m * P:(m + 1) * P],
                            rhs=h1T[:, j, :],
                            start=(j == 0), stop=(j == NM - 1),
                            tile_position=(0, 0))
                    if use_b2:
                        tmp = small.tile([P, HT], F32, tag="b2tmp",
                                         name="b2tmp")
                        nc.vector.tensor_scalar(tmp[:], ps[:],
                                                w_sb["b2"][:, m:m + 1], None,
                                                OP.add)
                        nc.vector.tensor_add(xT[:, m, tsl], xT[:, m, tsl],
                                             tmp[:])
                    else:
                        nc.vector.tensor_add(xT[:, m, tsl], xT[:, m, tsl],
                                             ps[:])

            def kv_half(w_sb, hT, kT, vloc, h):
                """K^T chunks (all heads, token half) + V (token chunks of h)."""
                tsl = slice(h * HT, (h + 1) * HT)
                for m in range(NJ):
                    ps = ps_s.tile([P, HT], F32, tag="s", name="kmm")
                    for j in range(NJ):
                        nc.tensor.matmul(
                            ps[:], lhsT=w_sb["wk"][:, j, m * P:(m + 1) * P],
                            rhs=hT[:, j, tsl],
                            start=(j == 0), stop=(j == NJ - 1),
                            tile_position=(0, 0))
                    nc.vector.tensor_copy(kT[:, m, tsl], ps[:])
                for mt in (2 * h, 2 * h + 1):
                    ps = ps_s.tile([P, D], F32, tag="s", name="vmm")
                    for j in range(NJ):
                        nc.tensor.matmul(
                            ps[:], lhsT=hT[:, j, mt * P:(mt + 1) * P],
                            rhs=w_sb["wv"][:, j, :],
                            start=(j == 0), stop=(j == NJ - 1),
                            tile_position=(0, 0))
                    nc.vector.tensor_copy(vloc[:, mt, :], ps[:])

            def q_half(w_sb, hT, qT, h):
                tsl = slice(h * HT, (h + 1) * HT)
                for m in range(NJ):
                    ps = ps_s.tile([P, HT], F32, tag="s", name="qmm")
                    for j in range(NJ):
                        nc.tensor.matmul(
                            ps[:], lhsT=w_sb["wq"][:, j, m * P:(m + 1) * P],
                            rhs=hT[:, j, tsl],
                            start=(j == 0), stop=(j == NJ - 1),
                            tile_position=(0, 0))
                    nc.vector.tensor_scalar(qT[:, m, tsl], ps[:],
                                            1.0 / math.sqrt(DK), None, OP.mult)

            def launch_ag(kT, vloc, h, l):
                """AG half h: K^T (all heads) + V for token half h."""
                tsl = slice(h * HT, (h + 1) * HT)
                kv_h = dram.tile([2, NJ * P * HT], BF16, tag=f"kv{h}",
                                 name=f"kv{h}_{l}")
                nc.sync.dma_start(
                    kv_h[0].rearrange("(j p t) -> p j t", p=P, j=NJ),
                    kT[:, :, tsl])
                nc.sync.dma_start(
                    kv_h[1].rearrange("(m p d) -> p m d", p=P, m=2),
                    vloc[:, 2 * h:2 * h + 2, :])
                ag_h = dram.tile([GROUP, 2, NJ * P * HT], BF16, tag=f"ag{h}",
                                 name=f"ag{h}_{l}")
                nc.gpsimd.collective_compute(
                    "AllGather", OP.bypass, replica_groups=REPLICA_GROUPS,
                    ins=[kv_h[:].opt()], outs=[ag_h[:].opt()],
                )
                return ag_h

            def attention(qT, ags, attnT):
                """Head pairs processed in groups of two, half-major: both
                pairs' AG-half-0 key chunks first (16 exp instructions of
                cover for the second AllGather), then both halves' remainder.
                Normalizes are deferred into the next group's first chunks so
                the av/z PSUM slots never head-of-line block the PE."""
                pending = []   # [(av, z, pair)] awaiting normalize

                def normalize():
                    av, z, pair = pending.pop()
                    # 1/Z per packed head row via fast custom-DVE reciprocal
                    rinv0 = small.tile([1, T], F32, tag="zrec0", bufs=1,
                                       name="zrec0")
                    nc.vector.reciprocal_approx_fast(rinv0[:], z[0:1, :])
                    rinv1 = small.tile([1, T], F32, tag="zrec1", bufs=1,
                                       name="zrec1")
                    nc.vector.reciprocal_approx_fast(rinv1[:], z[32:33, :])
                    bc = ps_z.tile([P, T], F32, tag="z", name="recbc")
                    nc.tensor.matmul(bc[0:DK, :], lhsT=ones_row_sb[0:1, 0:DK],
                                     rhs=rinv0[:], start=True, stop=True,
                                     tile_position=(0, 0))
                    nc.tensor.matmul(bc[DK:P, :], lhsT=ones_row_sb[0:1, 0:DK],
                                     rhs=rinv1[:], start=True, stop=True,
                                     tile_position=(0, 64))
                    bc_sb = small.tile([P, T], F32, tag="bc_sb", bufs=1, name="bc_sb")
                    nc.vector.tensor_copy(bc_sb[:], bc[:])
                    nc.vector.tensor_tensor(attnT[:, pair, :], av[:],
                                            bc_sb[:], OP.mult)

                for pg in range(H // 4):
                    pairs = (2 * pg, 2 * pg + 1)
                    kTps, vpss, avs, zs, nch = {}, {}, {}, {}, {}
                    for pair in pairs:
                        kTps[pair] = kvp.tile([P, S], BF16, tag="kTp", bufs=3,
                                              name=f"kTp{pair}")
                        vpss[pair] = {}
                        avs[pair] = ps_av.tile([P, T], F32, tag="av",
                                               name=f"av{pair}")
                        zs[pair] = ps_z.tile([P, T], F32, tag="z",
                                             name=f"z{pair}")
                        nch[pair] = 0
                    for h in range(2):
                        for pair in pairs:
                            kTp = kTps[pair]
                            for r in range(GROUP):
                                nc.sync.dma_start(
                                    kTp[:, r * T + h * HT:
                                        r * T + (h + 1) * HT],
                                    ags[h][r, 0]
                                    .rearrange("(j p t) -> p j t", p=P, j=NJ)
                                    [:, pair, :])
                            vp = kvp.tile([P, 2 * GROUP, P], BF16,
                                          tag=f"vp{h}", bufs=3,
                                          name=f"vp{h}_{pair}")
                            for r in range(GROUP):
                                nc.sync.dma_start(
                                    vp[:, 2 * r:2 * r + 2, :],
                                    ags[h][r, 1]
                                    .rearrange("(m p d) -> p m d", p=P, m=2)
                                    [:, :, pair * P:(pair + 1) * P])
                            vpss[pair][h] = vp
                        for pair in pairs:
                            kTp, av, z = kTps[pair], avs[pair], zs[pair]
                            vp = vpss[pair][h]
                            for r in range(GROUP):
                                for j in range(2):
                                    cl = 2 * h + j
                                    c = r * NJ + cl    # global key chunk
                                    pss = ps_s.tile([P, 2, T], F32, tag="s",
                                                    name="pss")
                                    nc.tensor.matmul(
                                        pss[:, 0, :],
                                        lhsT=kTp[0:DK, c * P:(c + 1) * P],
                                        rhs=qT[0:DK, pair, :],
                                        start=True, stop=True,
                                        tile_position=(0, 0))
                                    nc.tensor.matmul(
                                        pss[:, 1, :],
                                        lhsT=kTp[DK:P, c * P:(c + 1) * P],
                                        rhs=qT[DK:P, pair, :],
                                        start=True, stop=True,
                                        tile_position=(64, 0))
                                    ep = expp.tile([P, 2, T], BF16, tag="ep",
                                                   name="ep")
                                    nc.scalar.activation(
                                        ep[:], pss[:], AF.Exp,
                                        bias=kmask_sb[:, c:c + 1])
                                    if nch[pair] == 0 and pending:
                                        normalize()   # prev group, overlaps
                                    first = nch[pair] == 0
                                    last = nch[pair] == NC - 1
                                    nch[pair] += 1
                                    vtile = vp[:, 2 * r + j, :]
                                    nc.tensor.matmul(
                                        av[0:DK, :], lhsT=vtile[:, 0:DK],
                                        rhs=ep[:, 0, :], start=first,
                                        stop=last, tile_position=(0, 0),
                                        skip_group_check=True)
                                    nc.tensor.matmul(
                                        av[DK:P, :], lhsT=vtile[:, DK:P],
                                        rhs=ep[:, 1, :], start=first,
                                        stop=last, tile_position=(0, 64),
                                        skip_group_check=True)
                                    nc.tensor.matmul(
                                        z[0:1, :], lhsT=ones_bf_sb[:],
                                        rhs=ep[:, 0, :], start=first,
                                        stop=last, tile_position=(0, 0),
                                        skip_group_check=True)
                                    nc.tensor.matmul(
                                        z[32:33, :], lhsT=ones_bf_sb[:],
                                        rhs=ep[:, 1, :], start=first,
                                        stop=last, tile_position=(0, 32),
                                        skip_group_check=True)
                    for pair in pairs:
                        pending.append((avs[pair], zs[pair], pair))
                while pending:
                    normalize()

            # ---------------- prologue: LN1(0) + KV(0)/AG(0)/Q(0) ----------
            w_sb = load_weights(0)
            hT = work.tile([P, NJ, T], BF16, tag="hT", name="hT")
            kT = work.tile([P, NJ, T], BF16, tag="kT", name="kT")
            vloc = work.tile([P, NJ, D], BF16, tag="vloc", name="vloc")
            qT = work.tile([P, NJ, T], BF16, tag="qT", name="qT")
            ags = []
            for h in range(2):
                layernorm(hT[:, :, h * HT:(h + 1) * HT], 0, use_ln1, h)
                kv_half(w_sb, hT, kT, vloc, h)
                ags.append(launch_ag(kT, vloc, h, 0))
                q_half(w_sb, hT, qT, h)

            # ---------------- layers ----------------
            for l in range(L):
                attnT = work.tile([P, NJ, T], BF16, tag="attnT", name="attnT")
                attention(qT, ags, attnT)

                if l + 1 < L:
                    w_next = load_weights(l + 1)
                    hT = work.tile([P, NJ, T], BF16, tag="hT", name="hT")
                    kT = work.tile([P, NJ, T], BF16, tag="kT", name="kT")
                    vloc = work.tile([P, NJ, D], BF16, tag="vloc",
                                     name="vloc")
                    qT = work.tile([P, NJ, T], BF16, tag="qT", name="qT")
                    ags = []

                for h in range(2):
                    wo_half(w_sb, attnT, h)
                    gT = work.tile([P, NJ, HT], BF16, tag="gT", name="gT")
                    layernorm(gT, 2 * l + 1, use_ln2, h)
                    ffn_half(w_sb, gT, h)
                    if l + 1 < L:
                        layernorm(hT[:, :, h * HT:(h + 1) * HT],
                                  2 * (l + 1), use_ln1, h)
                        kv_half(w_next, hT, kT, vloc, h)
                        ags.append(launch_ag(kT, vloc, h, l + 1))
                        q_half(w_next, hT, qT, h)

                if l + 1 < L:
                    w_sb = w_next

            # ---------------- final LN + output ----------------
            outT = work.tile([P, NJ, T], F32, tag="outT", name="outT")
            layernorm(outT, 2 * L, use_fn)
            out_sb = work.tile([P, NJ, D], F32, tag="lnt", name="out_sb")
            for dj in range(NJ):
                for tj in range(NJ):
                    pst = ps_s.tile([P, P], F32, tag="s", name="otr")
                    nc.tensor.transpose(pst[:], outT[:, dj, tj * P:(tj + 1) * P],
                                        ident_sb[:])
                    nc.vector.tensor_copy(out_sb[:, tj, dj * P:(dj + 1) * P],
                                          pst[:])
            nc.sync.dma_start(y.rearrange("(j p) d -> p j d", p=P), out_sb[:])
        import sys, time
        print(f"[build] body traced {time.time():.0f}", file=sys.stderr, flush=True)

    print(f"[build] tile scheduled {time.time():.0f}", file=sys.stderr, flush=True)
    nc.compile()
    print(f"[build] bacc compiled {time.time():.0f}", file=sys.stderr, flush=True)
    return nc


# ---------------------------------------------------------------------------
_CACHE = {}


def _get_kernel(flags):
    if flags not in _CACHE:
        _CACHE[flags] = build_kernel(*flags)
    return _CACHE[flags]


def _chunkP(a):
    """[..., n*P, m] -> [..., P, n, m] with dim = n_idx*P + p."""
    a = np.asarray(a)
    *lead, npm, m = a.shape
    n = npm // P
    return np.ascontiguousarray(a.reshape(*lead, n, P, m).swapaxes(-3, -2))


def kernel(**inputs):
    src = np.asarray(inputs["src"]).astype(np.int64)
    emb = np.asarray(inputs["emb"], np.float32)
    pe = np.asarray(inputs["pe"], np.float32)
    W = {k: np.asarray(inputs[k], np.float32)
         for k in ("WQ", "WK", "WV", "WO", "W1", "W2", "b1", "b2",
                   "ln1_s", "ln1_b", "ln2_s", "ln2_b", "fn_s", "fn_b")}

    use_b1 = bool(np.any(W["b1"] != 0.0))
    use_b2 = bool(np.any(W["b2"] != 0.0))
    use_ln1 = bool(np.any(W["ln1_s"] != 1.0) or np.any(W["ln1_b"] != 0.0))
    use_ln2 = bool(np.any(W["ln2_s"] != 1.0) or np.any(W["ln2_b"] != 0.0))
    use_fn = bool(np.any(W["fn_s"] != 1.0) or np.any(W["fn_b"] != 0.0))
    nc = _get_kernel((use_b1, use_b2, use_ln1, use_ln2, use_fn))

    def perD(a):  # [L, D] -> [L, P, NJ] (d = j*P + p)
        a = np.asarray(a, np.float32)
        return np.ascontiguousarray(a.reshape(-1, NJ, P).swapaxes(-2, -1))

    lnsb = np.zeros((P, 2 * L + 1, 2, NJ), np.float32)
    for l in range(L):
        lnsb[:, 2 * l, 0] = perD(W["ln1_s"])[l]
        lnsb[:, 2 * l, 1] = perD(W["ln1_b"])[l]
        lnsb[:, 2 * l + 1, 0] = perD(W["ln2_s"])[l]
        lnsb[:, 2 * l + 1, 1] = perD(W["ln2_b"])[l]
    lnsb[:, 2 * L, 0] = perD(W["fn_s"][None])[0]
    lnsb[:, 2 * L, 1] = perD(W["fn_b"][None])[0]

    sel = np.zeros((2, P), np.float32)
    sel[0, :DK] = 1.0
    sel[1, DK:] = 1.0

    shared = {
        "emb": emb,
        "wq": _chunkP(W["WQ"]).astype(ml_dtypes.bfloat16),
        "wk": _chunkP(W["WK"]).astype(ml_dtypes.bfloat16),
        "wv": _chunkP(W["WV"]).astype(ml_dtypes.bfloat16),
        "wo": _chunkP(W["WO"]).astype(ml_dtypes.bfloat16),
        "w1": _chunkP(W["W1"]).astype(ml_dtypes.bfloat16),
        "w2": _chunkP(W["W2"]).astype(ml_dtypes.bfloat16),
        "b1t": np.ascontiguousarray(W["b1"].reshape(L, NM, P).swapaxes(1, 2)),
        "b2t": np.ascontiguousarray(W["b2"].reshape(L, NJ, P).swapaxes(1, 2)),
        "lnsb": lnsb,
        "ones_bf": np.ones((P, 1), ml_dtypes.bfloat16),
        "oneD": np.full((P, 1), 1.0 / D, np.float32),
        "ones_row": np.ones((1, P), np.float32),
        "sel2": sel,
        "ident": np.eye(P, dtype=np.float32),
    }

    in_maps = []
    for c in range(NCORES):
        b, sc = c // GROUP, c % GROUP
        ids = src[b, sc * T:(sc + 1) * T].astype(np.int32)
        peslice = pe[sc * T:(sc + 1) * T, :]                   # [T, D]
        pett = np.ascontiguousarray(
            peslice.T.reshape(NJ, P, T).swapaxes(0, 1))        # [P, NJ, T]
        maskn = np.where(src[b] == PAD, NEG, 0.0).astype(np.float32)
        m = dict(shared)
        m["tok"] = np.ascontiguousarray(ids.reshape(NJ, P).T)
        m["pet"] = pett.astype(np.float32)
        m["kmaskn"] = np.ascontiguousarray(maskn.reshape(NC, P).T)
        in_maps.append(m)

    res = run_bass_kernel_spmd(nc, in_maps, core_ids=list(range(NCORES)))
    out = np.empty((B, S, D), np.float32)
    for c in range(NCORES):
        b, sc = c // GROUP, c % GROUP
        out[b, sc * T:(sc + 1) * T, :] = res.results[c]["y"]
    return out


# revision 20
# speedup vs baseline: 1.2160x; 1.1119x over previous
"""Trainium2 Bass kernel for a 6-layer pre-LN transformer encoder (nn_Encoder).

Distribution: tokens sharded 8 ways (core c -> batch c//4, seq chunk c%4 of 512
tokens).  Per layer, each core computes K^T/V projections for its own tokens,
AllGathers them (bf16) within its batch group of 4 cores, and computes
attention for its 512 query tokens over the full 2048-key sequence.
Projections / FFN / LayerNorms are purely local to the token shard.

The residual stream is kept transposed (x.T: [D, tok]) so every matmul
consumes operands in natural layout with no per-layer transposes:
  q.T/k.T = W.T @ h.T    (lhsT = W chunk,   rhs = h.T chunk)
  v       = h @ W        (lhsT = h.T chunk, rhs = W chunk)
  scores.T[k,q]          (lhsT = K.T chunk, rhs = q.T head)  [2 heads row-packed]
  attn.T  = V.T @ P.T    (lhsT = V chunk,   rhs = expP)      [2 heads col-packed]
  Z       = ones.T @ P.T                                     [2 heads col-packed]
  out.T   = WO.T @ attn.T; FFN analogous.
LayerNorm reductions over D (partition axis) use (1/D)-valued ones-matmuls;
per-token scalars are broadcast across partitions with K=1 ones matmuls.
rsqrt(var+eps) = Exp(-0.5 * Ln(var+eps)) keeps ACT on one table set.
The key-padding mask folds into the exp() activation bias (per-partition).

v2 pipeline: everything after attention (WO/LN2/FFN/LN1'/KV'/Q') is processed
in two token halves; the K/V AllGather for layer l+1 is split by token half
and launched at the end of each half's tail work, so the collective overlaps
the other half's FFN + the next attention's first chunks.  Softmax 1/Z uses a
single custom-DVE reciprocal_approx_fast over both packed head rows.
"""

import math

import numpy as np
import ml_dtypes

import concourse.bass as bass
import concourse.mybir as mybir
import concourse.tile as tile
from concourse import bacc
from concourse.bass_utils import run_bass_kernel_spmd

F32 = mybir.dt.float32
BF16 = mybir.dt.bfloat16
I32 = mybir.dt.int32
AF = mybir.ActivationFunctionType
OP = mybir.AluOpType

VOCAB, D, H, L, DFF, PAD = 32000, 512, 8, 6, 2048, 0
B, S = 2, 2048
DK = D // H          # 64
P = 128
T = 512              # tokens per core
HT = T // 2          # tokens per half (tail pipeline granularity)
NJ = D // P          # 4   D-chunks
NM = DFF // P        # 16  DFF-chunks
NC = S // P          # 16  key chunks per batch row
NCORES = 8
GROUP = 4            # cores per batch group
EPS = 1e-5
SQRT_D = math.sqrt(D)
NEG = -80.0          # mask bias: exp(s-80) is zero for practical purposes

REPLICA_GROUPS = [[0, 1, 2, 3], [4, 5, 6, 7]]


def _patch_act_tables():
    """Steer every Exp activation to natural_log_exp_and_others (which holds
    both exp and ln) so the table never reloads between LayerNorm (Ln) and
    attention softmax (Exp).  Only set *contents* are changed; set order (and
    hence act_func_set ids) is preserved."""
    import concourse.hw_specs as hw_specs

    if getattr(hw_specs, "_exp_set_patch", False):
        return
    orig = hw_specs.get_activation_tables

    import functools

    @functools.cache
    def patched(module_arch):
        tabs = {k: set(v) for k, v in orig(module_arch).items()}
        if "natural_log_exp_and_others" in tabs:
            for name, fns in tabs.items():
                if name != "natural_log_exp_and_others":
                    fns.discard(AF.Exp)
        return tabs

    hw_specs.get_activation_tables = patched
    hw_specs._exp_set_patch = True
    bacc.get_activation_tables = patched


def build_kernel(use_b1, use_b2, use_ln1, use_ln2, use_fn):
    _patch_act_tables()
    nc = bacc.Bacc("TRN2", target_bir_lowering=False, debug=False,
                   num_devices=NCORES)

    # register EPS as a const AP so activation(bias=EPS) works
    _eps_t = nc.alloc_sbuf_tensor("const-float32-eps", [P, 1], F32)
    nc.gpsimd.memset(_eps_t.ap(), EPS)
    nc.const_aps.aps[(F32, EPS)] = _eps_t.ap()

    # ---------------- parameters ----------------
    tok = nc.declare_dram_parameter("tok", [P, NJ], I32, isOutput=False)
    emb = nc.declare_dram_parameter("emb", [VOCAB, D], F32, isOutput=False)
    pet = nc.declare_dram_parameter("pet", [P, NJ, T], F32, isOutput=False)
    wq = nc.declare_dram_parameter("wq", [L, P, NJ, D], BF16, isOutput=False)
    wk = nc.declare_dram_parameter("wk", [L, P, NJ, D], BF16, isOutput=False)
    wv = nc.declare_dram_parameter("wv", [L, P, NJ, D], BF16, isOutput=False)
    wo = nc.declare_dram_parameter("wo", [L, P, NJ, D], BF16, isOutput=False)
    w1 = nc.declare_dram_parameter("w1", [L, P, NJ, DFF], BF16, isOutput=False)
    w2 = nc.declare_dram_parameter("w2", [L, P, NM, D], BF16, isOutput=False)
    b1t = nc.declare_dram_parameter("b1t", [L, P, NM], F32, isOutput=False)
    b2t = nc.declare_dram_parameter("b2t", [L, P, NJ], F32, isOutput=False)
    lnsb = nc.declare_dram_parameter("lnsb", [P, 2 * L + 1, 2, NJ], F32,
                                     isOutput=False)
    kmaskn = nc.declare_dram_parameter("kmaskn", [P, NC], F32, isOutput=False)
    ones_bf = nc.declare_dram_parameter("ones_bf", [P, 1], BF16, isOutput=False)
    oneD = nc.declare_dram_parameter("oneD", [P, 1], F32, isOutput=False)
    ones_row = nc.declare_dram_parameter("ones_row", [1, P], F32, isOutput=False)
    sel2 = nc.declare_dram_parameter("sel2", [2, P], F32, isOutput=False)
    ident = nc.declare_dram_parameter("ident", [P, P], F32, isOutput=False)
    y = nc.declare_dram_parameter("y", [T, D], F32, isOutput=True)

    use_lnp = use_ln1 or use_ln2 or use_fn

    with tile.TileContext(nc) as tc:
        with (
            tc.tile_pool(name="wpool", bufs=2) as wpool,
            tc.tile_pool(name="work", bufs=1) as work,
            tc.tile_pool(name="small", bufs=2) as small,
            tc.tile_pool(name="kv", bufs=3) as kvp,
            tc.tile_pool(name="expp", bufs=4) as expp,
            tc.tile_pool(name="const", bufs=1) as constp,
            tc.tile_pool(name="ps_s", bufs=2, space="PSUM") as ps_s,
            tc.tile_pool(name="ps_av", bufs=2, space="PSUM") as ps_av,
            tc.tile_pool(name="ps_z", bufs=2, space="PSUM") as ps_z,
            tc.tile_pool(name="dram", bufs=2, space="DRAM") as dram,
        ):
            # ---------------- constants / persistent state ----------------
            tok_sb = constp.tile([P, NJ], I32)
            nc.sync.dma_start(tok_sb[:], tok[:])
            ones_bf_sb = constp.tile([P, 1], BF16)
            nc.sync.dma_start(ones_bf_sb[:], ones_bf[:])
            oneD_sb = constp.tile([P, 1], F32)
            nc.sync.dma_start(oneD_sb[:], oneD[:])
            ones_row_sb = constp.tile([1, P], F32)
            nc.sync.dma_start(ones_row_sb[:], ones_row[:])
            ident_sb = constp.tile([P, P], F32)
            nc.sync.dma_start(ident_sb[:], ident[:])
            pet_sb = constp.tile([P, NJ, T], F32)
            nc.sync.dma_start(pet_sb[:], pet[:])
            kmask_sb = constp.tile([P, NC], F32)
            nc.sync.dma_start(kmask_sb[:], kmaskn[:])
            lnp_sb = None
            if use_lnp:
                lnp_sb = constp.tile([P, 2 * L + 1, 2, NJ], F32, name="lnp")
                nc.sync.dma_start(lnp_sb[:], lnsb[:])

            xT = constp.tile([P, NJ, T], F32, name="xT")  # residual stream x.T

            def load_weights(l, what=("wq", "wk", "wv", "wo", "w1", "w2")):
                t = {}
                if "wq" in what:
                    t["wq"] = wpool.tile([P, NJ, D], BF16, tag="wq", name="wq")
                    nc.sync.dma_start(t["wq"][:], wq[l])
                if "wk" in what:
                    t["wk"] = wpool.tile([P, NJ, D], BF16, tag="wk", name="wk")
                    nc.sync.dma_start(t["wk"][:], wk[l])
                if "wv" in what:
                    t["wv"] = wpool.tile([P, NJ, D], BF16, tag="wv", name="wv")
                    nc.sync.dma_start(t["wv"][:], wv[l])
                if "wo" in what:
                    t["wo"] = wpool.tile([P, NJ, D], BF16, tag="wo", name="wo")
                    nc.sync.dma_start(t["wo"][:], wo[l])
                if "w1" in what:
                    t["w1"] = wpool.tile([P, NJ, DFF], BF16, tag="w1",
                                         name="w1")
                    nc.sync.dma_start(t["w1"][:], w1[l])
                    if use_b1:
                        t["b1"] = wpool.tile([P, NM], F32, tag="b1", name="b1")
                        nc.sync.dma_start(t["b1"][:], b1t[l])
                if "w2" in what:
                    t["w2"] = wpool.tile([P, NM, D], BF16, tag="w2", bufs=1,
                                         name="w2")
                    nc.sync.dma_start(t["w2"][:], w2[l])
                    if use_b2:
                        t["b2"] = wpool.tile([P, NJ], F32, tag="b2", name="b2")
                        nc.sync.dma_start(t["b2"][:], b2t[l])
                return t

            # ---------------- embedding gather + transpose ----------------
            for j in range(NJ):
                ex = small.tile([P, D], F32, tag="embx", bufs=1)
                nc.gpsimd.indirect_dma_start(
                    out=ex[:],
                    out_offset=None,
                    in_=emb[:, :],
                    in_offset=bass.IndirectOffsetOnAxis(ap=tok_sb[:, j:j + 1],
                                                        axis=0),
                )
                for dj in range(NJ):
                    pst = ps_s.tile([P, P], F32, tag="s")
                    nc.tensor.transpose(pst[:], ex[:, dj * P:(dj + 1) * P],
                                        ident_sb[:])
                    # x.T = emb.T * sqrt(D) + pe.T
                    sl = xT[:, dj, j * P:(j + 1) * P]
                    nc.vector.tensor_scalar(sl, pst[:], SQRT_D, None, OP.mult)
                    nc.vector.tensor_add(sl, sl, pet_sb[:, dj, j * P:(j + 1) * P])

            # ---------------- helpers ----------------
            def layernorm(out_t, param_idx, use_params, h=None):
                """LayerNorm over D (partition axis) of xT -> out_t [P,NJ,*].

                h=None: all T tokens; h=0/1: token half (HT columns)."""
                if h is None:
                    tsl = slice(0, T)
                    n = T
                else:
                    tsl = slice(h * HT, (h + 1) * HT)
                    n = HT
                sq = work.tile([P, NJ, n], F32, tag="lnt", name="sq")
                nc.vector.tensor_tensor(sq[:], xT[:, :, tsl], xT[:, :, tsl],
                                        OP.mult)
                st = ps_s.tile([P, n], F32, tag="s", name="st")
                for j in range(NJ):
                    nc.tensor.matmul(st[0:1, :], lhsT=oneD_sb[:],
                                     rhs=xT[:, j, tsl],
                                     start=(j == 0), stop=(j == NJ - 1),
                                     tile_position=(0, 0))
                    nc.tensor.matmul(st[32:33, :], lhsT=oneD_sb[:],
                                     rhs=sq[:, j, :],
                                     start=(j == 0), stop=(j == NJ - 1),
                                     tile_position=(0, 32))
                mu_sb = small.tile([1, n], F32, tag="mu_sb", bufs=1)
                nc.vector.tensor_copy(mu_sb[:], st[0:1, :])
                ex2_sb = small.tile([1, n], F32, tag="ex2_sb", bufs=1)
                nc.vector.tensor_copy(ex2_sb[:], st[32:33, :])
                mu = mu_sb[:]
                var = small.tile([1, n], F32, tag="var", bufs=1)
                nc.vector.tensor_tensor(var[:], mu, mu, OP.mult)
                nc.vector.tensor_tensor(var[:], ex2_sb[:], var[:], OP.subtract)
                lnv = small.tile([1, n], F32, tag="lnv", bufs=1)
                nc.scalar.activation(lnv[:], var[:], AF.Ln, bias=EPS)
                rinv = small.tile([1, n], F32, tag="rinv", bufs=1)
                nc.scalar.activation(rinv[:], lnv[:], AF.Exp, scale=-0.5)
                bc_mu = ps_s.tile([P, n], F32, tag="s", name="bc_mu")
                nc.tensor.matmul(bc_mu[:], lhsT=ones_row_sb[:], rhs=mu,
                                 start=True, stop=True, tile_position=(0, 0))
                bc_ri = ps_s.tile([P, n], F32, tag="s", name="bc_ri")
                nc.tensor.matmul(bc_ri[:], lhsT=ones_row_sb[:], rhs=rinv[:],
                                 start=True, stop=True, tile_position=(0, 0))
                t1 = work.tile([P, NJ, n], F32, tag="lnt", name="lnt")
                nc.vector.tensor_tensor(
                    t1[:], xT[:, :, tsl],
                    bc_mu[:, None, :].to_broadcast([P, NJ, n]),
                    OP.subtract)
                nc.vector.tensor_tensor(
                    out_t[:], t1[:], bc_ri[:, None, :].to_broadcast([P, NJ, n]),
                    OP.mult)
                if use_params:
                    for j in range(NJ):
                        nc.vector.tensor_scalar(
                            out_t[:, j, :], out_t[:, j, :],
                            lnp_sb[:, param_idx, 0, j:j + 1],
                            lnp_sb[:, param_idx, 1, j:j + 1],
                            OP.mult, OP.add)

            # ---- tail-half building blocks (token half h of tokens tsl) ----
            def wo_half(w_sb, attnT, h):
                tsl = slice(h * HT, (h + 1) * HT)
                for m in range(NJ):
                    ps = ps_s.tile([P, HT], F32, tag="s", name="omm")
                    for j in range(NJ):
                        nc.tensor.matmul(
                            ps[:], lhsT=w_sb["wo"][:, j, m * P:(m + 1) * P],
                            rhs=attnT[:, j, tsl],
                            start=(j == 0), stop=(j == NJ - 1),
                            tile_position=(0, 0))
                    nc.vector.tensor_add(xT[:, m, tsl], xT[:, m, tsl], ps[:])

            def ffn_half(w_sb, gT, h):
                tsl = slice(h * HT, (h + 1) * HT)
                h1T = work.tile([P, NM, HT], BF16, tag="h1T", name="h1T")
                for m in range(NM):
                    ps = ps_s.tile([P, HT], F32, tag="s", name="f1mm")
                    for j in range(NJ):
                        nc.tensor.matmul(
                            ps[:], lhsT=w_sb["w1"][:, j, m * P:(m + 1) * P],
                            rhs=gT[:, j, :],
                            start=(j == 0), stop=(j == NJ - 1),
                            tile_position=(0, 0))
                    if use_b1:
                        nc.vector.tensor_scalar(h1T[:, m, :], ps[:],
                                                w_sb["b1"][:, m:m + 1], 0.0,
                                                OP.add, OP.max)
                    else:
                        nc.vector.tensor_scalar(h1T[:, m, :], ps[:], 0.0, None,
                                                OP.max)
                for m in range(NJ):
                    ps = ps_s.tile([P, HT], F32, tag="s", name="f2mm")
                    for j in range(NM):
                        nc.tensor.matmul(
                            ps[:], lhsT=w_sb["w2"][:, j, m * P:(m + 1) * P],
                            rhs=h1T[:, j, :],
                            start=(j == 0), stop=(j == NM - 1),
                            tile_position=(0, 0))
                    if use_b2:
                        tmp = small.tile([P, HT], F32, tag="b2tmp",
                                         name="b2tmp")
                        nc.vector.tensor_scalar(tmp[:], ps[:],
                                                w_sb["b2"][:, m:m + 1], None,
                                                OP.add)
                        nc.vector.tensor_add(xT[:, m, tsl], xT[:, m, tsl],
                                             tmp[:])
                    else:
                        nc.vector.tensor_add(xT[:, m, tsl], xT[:, m, tsl],
                                             ps[:])

            def kv_half(w_sb, hT, kT, vloc, h):
                """K^T chunks (all heads, token half) + V (token chunks of h)."""
                tsl = slice(h * HT, (h + 1) * HT)
                for m in range(NJ):
                    ps = ps_s.tile([P, HT], F32, tag="s", name="kmm")
                    for j in range(NJ):
                        nc.tensor.matmul(
                            ps[:], lhsT=w_sb["wk"][:, j, m * P:(m + 1) * P],
                            rhs=hT[:, j, tsl],
                            start=(j == 0), stop=(j == NJ - 1),
                            tile_position=(0, 0))
                    nc.vector.tensor_copy(kT[:, m, tsl], ps[:])
                for mt in (2 * h, 2 * h + 1):
                    ps = ps_s.tile([P, D], F32, tag="s", name="vmm")
                    for j in range(NJ):
                        nc.tensor.matmul(
                            ps[:], lhsT=hT[:, j, mt * P:(mt + 1) * P],
                            rhs=w_sb["wv"][:, j, :],
                            start=(j == 0), stop=(j == NJ - 1),
                            tile_position=(0, 0))
                    nc.vector.tensor_copy(vloc[:, mt, :], ps[:])

            def q_half(w_sb, hT, qT, h):
                tsl = slice(h * HT, (h + 1) * HT)
                for m in range(NJ):
                    ps = ps_s.tile([P, HT], F32, tag="s", name="qmm")
                    for j in range(NJ):
                        nc.tensor.matmul(
                            ps[:], lhsT=w_sb["wq"][:, j, m * P:(m + 1) * P],
                            rhs=hT[:, j, tsl],
                            start=(j == 0), stop=(j == NJ - 1),
                            tile_position=(0, 0))
                    nc.vector.tensor_scalar(qT[:, m, tsl], ps[:],
                                            1.0 / math.sqrt(DK), None, OP.mult)

            def launch_ag(kT, vloc, h, l):
                """AG half h: K^T (all heads) + V for token half h."""
                tsl = slice(h * HT, (h + 1) * HT)
                kv_h = dram.tile([2, NJ * P * HT], BF16, tag=f"kv{h}",
                                 name=f"kv{h}_{l}")
                nc.sync.dma_start(
                    kv_h[0].rearrange("(j p t) -> p j t", p=P, j=NJ),
                    kT[:, :, tsl])
                nc.sync.dma_start(
                    kv_h[1].rearrange("(m p d) -> p m d", p=P, m=2),
                    vloc[:, 2 * h:2 * h + 2, :])
                ag_h = dram.tile([GROUP, 2, NJ * P * HT], BF16, tag=f"ag{h}",
                                 name=f"ag{h}_{l}")
                nc.gpsimd.collective_compute(
                    "AllGather", OP.bypass, replica_groups=REPLICA_GROUPS,
                    ins=[kv_h[:].opt()], outs=[ag_h[:].opt()],
                )
                return ag_h

            def attention(qT, ags, attnT):
                """Head pairs processed in groups of two, half-major: both
                pairs' AG-half-0 key chunks first (16 exp instructions of
                cover for the second AllGather), then both halves' remainder.
                Normalizes are deferred into the next group's first chunks so
                the av/z PSUM slots never head-of-line block the PE."""
                pending = []   # [(av, z, pair)] awaiting normalize

                def normalize():
                    av, z, pair = pending.pop()
                    # 1/Z per packed head row via fast custom-DVE reciprocal
                    rinv0 = small.tile([1, T], F32, tag="zrec0", bufs=1,
                                       name="zrec0")
                    nc.vector.reciprocal_approx_fast(rinv0[:], z[0:1, :])
                    rinv1 = small.tile([1, T], F32, tag="zrec1", bufs=1,
                                       name="zrec1")
                    nc.vector.reciprocal_approx_fast(rinv1[:], z[32:33, :])
                    bc = ps_z.tile([P, T], F32, tag="z", name="recbc")
                    nc.tensor.matmul(bc[0:DK, :], lhsT=ones_row_sb[0:1, 0:DK],
                                     rhs=rinv0[:], start=True, stop=True,
                                     tile_position=(0, 0))
                    nc.tensor.matmul(bc[DK:P, :], lhsT=ones_row_sb[0:1, 0:DK],
                                     rhs=rinv1[:], start=True, stop=True,
                                     tile_position=(0, 64))
                    bc_sb = small.tile([P, T], F32, tag="bc_sb", bufs=1, name="bc_sb")
                    nc.vector.tensor_copy(bc_sb[:], bc[:])
                    nc.vector.tensor_tensor(attnT[:, pair, :], av[:],
                                            bc_sb[:], OP.mult)

                for pg in range(H // 4):
                    pairs = (2 * pg, 2 * pg + 1)
                    kTps, vpss, avs, zs, nch = {}, {}, {}, {}, {}
                    for pair in pairs:
                        kTps[pair] = kvp.tile([P, S], BF16, tag="kTp", bufs=3,
                                              name=f"kTp{pair}")
                        vpss[pair] = {}
                        avs[pair] = ps_av.tile([P, T], F32, tag="av",
                                               name=f"av{pair}")
                        zs[pair] = ps_z.tile([P, T], F32, tag="z",
                                             name=f"z{pair}")
                        nch[pair] = 0
                    for h in range(2):
                        for pair in pairs:
                            kTp = kTps[pair]
                            for r in range(GROUP):
                                nc.sync.dma_start(
                                    kTp[:, r * T + h * HT:
                                        r * T + (h + 1) * HT],
                                    ags[h][r, 0]
                                    .rearrange("(j p t) -> p j t", p=P, j=NJ)
                                    [:, pair, :])
                            vp = kvp.tile([P, 2 * GROUP, P], BF16,
                                          tag=f"vp{h}", bufs=3,
                                          name=f"vp{h}_{pair}")
                            for r in range(GROUP):
                                nc.sync.dma_start(
                                    vp[:, 2 * r:2 * r + 2, :],
                                    ags[h][r, 1]
                                    .rearrange("(m p d) -> p m d", p=P, m=2)
                                    [:, :, pair * P:(pair + 1) * P])
                            vpss[pair][h] = vp
                        for pair in pairs:
                            kTp, av, z = kTps[pair], avs[pair], zs[pair]
                            vp = vpss[pair][h]
                            for r in range(GROUP):
                                for j in range(2):
                                    cl = 2 * h + j
                                    c = r * NJ + cl    # global key chunk
                                    pss = ps_s.tile([P, 2, T], F32, tag="s",
                                                    name="pss")
                                    nc.tensor.matmul(
                                        pss[:, 0, :],
                                        lhsT=kTp[0:DK, c * P:(c + 1) * P],
                                        rhs=qT[0:DK, pair, :],
                                        start=True, stop=True,
                                        tile_position=(0, 0))
                                    nc.tensor.matmul(
                                        pss[:, 1, :],
                                        lhsT=kTp[DK:P, c * P:(c + 1) * P],
                                        rhs=qT[DK:P, pair, :],
                                        start=True, stop=True,
                                        tile_position=(64, 0))
                                    ep = expp.tile([P, 2, T], BF16, tag="ep",
                                                   name="ep")
                                    nc.scalar.activation(
                                        ep[:], pss[:], AF.Exp,
                                        bias=kmask_sb[:, c:c + 1])
                                    if nch[pair] == 0 and pending:
                                        normalize()   # prev group, overlaps
                                    first = nch[pair] == 0
                                    last = nch[pair] == NC - 1
                                    nch[pair] += 1
                                    vtile = vp[:, 2 * r + j, :]
                                    nc.tensor.matmul(
                                        av[0:DK, :], lhsT=vtile[:, 0:DK],
                                        rhs=ep[:, 0, :], start=first,
                                        stop=last, tile_position=(0, 0),
                                        skip_group_check=True)
                                    nc.tensor.matmul(
                                        av[DK:P, :], lhsT=vtile[:, DK:P],
                                        rhs=ep[:, 1, :], start=first,
                                        stop=last, tile_position=(0, 64),
                                        skip_group_check=True)
                                    nc.tensor.matmul(
                                        z[0:1, :], lhsT=ones_bf_sb[:],
                                        rhs=ep[:, 0, :], start=first,
                                        stop=last, tile_position=(0, 0),
                                        skip_group_check=True)
                                    nc.tensor.matmul(
                                        z[32:33, :], lhsT=ones_bf_sb[:],
                                        rhs=ep[:, 1, :], start=first,
                                        stop=last, tile_position=(0, 32),
                                        skip_group_check=True)
                    for pair in pairs:
                        pending.append((avs[pair], zs[pair], pair))
                while pending:
                    normalize()

            # ---------------- prologue: LN1(0) + KV(0)/AG(0)/Q(0) ----------
            w_sb = load_weights(0)
            hT = work.tile([P, NJ, T], BF16, tag="hT", name="hT")
            kT = work.tile([P, NJ, T], BF16, tag="kT", name="kT")
            vloc = work.tile([P, NJ, D], BF16, tag="vloc", name="vloc")
            qT = work.tile([P, NJ, T], BF16, tag="qT", name="qT")
            ags = []
            for h in range(2):
                layernorm(hT[:, :, h * HT:(h + 1) * HT], 0, use_ln1, h)
                kv_half(w_sb, hT, kT, vloc, h)
                ags.append(launch_ag(kT, vloc, h, 0))
                q_half(w_sb, hT, qT, h)

            # ---------------- layers ----------------
            for l in range(L):
                attnT = work.tile([P, NJ, T], BF16, tag="attnT", name="attnT")
                attention(qT, ags, attnT)

                if l + 1 < L:
                    w_next = load_weights(l + 1)
                    hT = work.tile([P, NJ, T], BF16, tag="hT", name="hT")
                    kT = work.tile([P, NJ, T], BF16, tag="kT", name="kT")
                    vloc = work.tile([P, NJ, D], BF16, tag="vloc",
                                     name="vloc")
                    qT = work.tile([P, NJ, T], BF16, tag="qT", name="qT")
                    ags = []

                for h in range(2):
                    wo_half(w_sb, attnT, h)
                    gT = work.tile([P, NJ, HT], BF16, tag="gT", name="gT")
                    layernorm(gT, 2 * l + 1, use_ln2, h)
                    ffn_half(w_sb, gT, h)
                    if l + 1 < L:
                        layernorm(hT[:, :, h * HT:(h + 1) * HT],
                                  2 * (l + 1), use_ln1, h)
                        kv_half(w_next, hT, kT, vloc, h)
                        ags.append(launch_ag(kT, vloc, h, l + 1))
                        q_half(w_next, hT, qT, h)

                if l + 1 < L:
                    w_sb = w_next

            # ---------------- final LN + output ----------------
            outT = work.tile([P, NJ, T], F32, tag="outT", name="outT")
            layernorm(outT, 2 * L, use_fn)
            out_sb = work.tile([P, NJ, D], F32, tag="lnt", name="out_sb")
            for dj in range(NJ):
                for tj in range(NJ):
                    pst = ps_s.tile([P, P], F32, tag="s", name="otr")
                    nc.tensor.transpose(pst[:], outT[:, dj, tj * P:(tj + 1) * P],
                                        ident_sb[:])
                    nc.vector.tensor_copy(out_sb[:, tj, dj * P:(dj + 1) * P],
                                          pst[:])
            nc.sync.dma_start(y.rearrange("(j p) d -> p j d", p=P), out_sb[:])
        import sys, time
        print(f"[build] body traced {time.time():.0f}", file=sys.stderr, flush=True)

    print(f"[build] tile scheduled {time.time():.0f}", file=sys.stderr, flush=True)
    nc.compile()
    print(f"[build] bacc compiled {time.time():.0f}", file=sys.stderr, flush=True)
    return nc


# ---------------------------------------------------------------------------
_CACHE = {}


def _get_kernel(flags):
    if flags not in _CACHE:
        _CACHE[flags] = build_kernel(*flags)
    return _CACHE[flags]


def _chunkP(a):
    """[..., n*P, m] -> [..., P, n, m] with dim = n_idx*P + p."""
    a = np.asarray(a)
    *lead, npm, m = a.shape
    n = npm // P
    return np.ascontiguousarray(a.reshape(*lead, n, P, m).swapaxes(-3, -2))


def kernel(**inputs):
    src = np.asarray(inputs["src"]).astype(np.int64)
    emb = np.asarray(inputs["emb"], np.float32)
    pe = np.asarray(inputs["pe"], np.float32)
    W = {k: np.asarray(inputs[k], np.float32)
         for k in ("WQ", "WK", "WV", "WO", "W1", "W2", "b1", "b2",
                   "ln1_s", "ln1_b", "ln2_s", "ln2_b", "fn_s", "fn_b")}

    use_b1 = bool(np.any(W["b1"] != 0.0))
    use_b2 = bool(np.any(W["b2"] != 0.0))
    use_ln1 = bool(np.any(W["ln1_s"] != 1.0) or np.any(W["ln1_b"] != 0.0))
    use_ln2 = bool(np.any(W["ln2_s"] != 1.0) or np.any(W["ln2_b"] != 0.0))
    use_fn = bool(np.any(W["fn_s"] != 1.0) or np.any(W["fn_b"] != 0.0))
    nc = _get_kernel((use_b1, use_b2, use_ln1, use_ln2, use_fn))

    def perD(a):  # [L, D] -> [L, P, NJ] (d = j*P + p)
        a = np.asarray(a, np.float32)
        return np.ascontiguousarray(a.reshape(-1, NJ, P).swapaxes(-2, -1))

    lnsb = np.zeros((P, 2 * L + 1, 2, NJ), np.float32)
    for l in range(L):
        lnsb[:, 2 * l, 0] = perD(W["ln1_s"])[l]
        lnsb[:, 2 * l, 1] = perD(W["ln1_b"])[l]
        lnsb[:, 2 * l + 1, 0] = perD(W["ln2_s"])[l]
        lnsb[:, 2 * l + 1, 1] = perD(W["ln2_b"])[l]
    lnsb[:, 2 * L, 0] = perD(W["fn_s"][None])[0]
    lnsb[:, 2 * L, 1] = perD(W["fn_b"][None])[0]

    sel = np.zeros((2, P), np.float32)
    sel[0, :DK] = 1.0
    sel[1, DK:] = 1.0

    shared = {
        "emb": emb,
        "wq": _chunkP(W["WQ"]).astype(ml_dtypes.bfloat16),
        "wk": _chunkP(W["WK"]).astype(ml_dtypes.bfloat16),
        "wv": _chunkP(W["WV"]).astype(ml_dtypes.bfloat16),
        "wo": _chunkP(W["WO"]).astype(ml_dtypes.bfloat16),
        "w1": _chunkP(W["W1"]).astype(ml_dtypes.bfloat16),
        "w2": _chunkP(W["W2"]).astype(ml_dtypes.bfloat16),
        "b1t": np.ascontiguousarray(W["b1"].reshape(L, NM, P).swapaxes(1, 2)),
        "b2t": np.ascontiguousarray(W["b2"].reshape(L, NJ, P).swapaxes(1, 2)),
        "lnsb": lnsb,
        "ones_bf": np.ones((P, 1), ml_dtypes.bfloat16),
        "oneD": np.full((P, 1), 1.0 / D, np.float32),
        "ones_row": np.ones((1, P), np.float32),
        "sel2": sel,
        "ident": np.eye(P, dtype=np.float32),
    }

    in_maps = []
    for c in range(NCORES):
        b, sc = c // GROUP, c % GROUP
        ids = src[b, sc * T:(sc + 1) * T].astype(np.int32)
        peslice = pe[sc * T:(sc + 1) * T, :]                   # [T, D]
        pett = np.ascontiguousarray(
            peslice.T.reshape(NJ, P, T).swapaxes(0, 1))        # [P, NJ, T]
        maskn = np.where(src[b] == PAD, NEG, 0.0).astype(np.float32)
        m = dict(shared)
        m["tok"] = np.ascontiguousarray(ids.reshape(NJ, P).T)
        m["pet"] = pett.astype(np.float32)
        m["kmaskn"] = np.ascontiguousarray(maskn.reshape(NC, P).T)
        in_maps.append(m)

    res = run_bass_kernel_spmd(nc, in_maps, core_ids=list(range(NCORES)))
    out = np.empty((B, S, D), np.float32)
    for c in range(NCORES):
        b, sc = c // GROUP, c % GROUP
        out[b, sc * T:(sc + 1) * T, :] = res.results[c]["y"]
    return out
